# revision 30
# baseline (speedup 1.0000x reference)
"""Trainium2 Bass kernel for nn_ATTLayer (attention pooling).

Reference computation (full input [64, 512, 1024] fp32):
    wb    = attention_w + attention_b          # [1024, 256] (b broadcast over rows)
    u_t   = tanh(inputs @ wb)                  # [64, 512, 256]
    logit = u_t @ attention_u                  # [64, 512]
    w     = softmax(logit, axis=1)             # softmax over seq
    out   = sum_s w[:, s] * inputs[:, s, :]    # [64, 1024]

Sharding: data-parallel over batch — 8 batches per core on 8 NeuronCores, no
collectives. Tiny params (wb = W + b, u) are precomputed/replicated on host.

Host-side input prep (inside kernel()): x is cast to bf16 and uploaded in TWO
layouts — natural [b, s, h] (feeds the weighted sum) and block-transposed
[b, k, h_local, s] (feeds GEMM1, which contracts over h and therefore needs h
on SBUF partitions). Uploading the transposed copy replaces 256 on-chip PE
transposes + PSUM evacuations per core and runs at full DMA rate; measured
xbar DMA-transpose (~100 GB/s) and SWDGE cast-DMA (~200 MB/s) were far slower.

Per-core dataflow, per local batch b (software-pipelined; Tile schedules):
  1. DMA x^T tiles (sync HWDGE ring)  and  x natural (scalar HWDGE ring)
  2. GEMM1 (bf16, fp32 PSUM): psum[a_chunk, s] += wb[k, a_chunk].T @ x^T[k]
     over k = 8 h-chunks; tanh on ScalarE -> u_t^T bf16 tiles [a_local, s]
  3. logit^T [s_local, t] via N=2 matmuls (u padded with zero columns to
     satisfy the even-free-dim ISA rule); wt = exp(logit^T) on ScalarE.
     No max-subtraction: |logit| <= sum|u| ~ 20 so exp stays finite in fp32;
     the 1/sum normalization folds into the final output scale.
  4. softmax sum = ones.T @ wt (PE) -> reduce -> reciprocal (VectorE)
  5. weighted sum (emitted one batch late so PE never waits on the exp):
     psum[1, h] += wt[:, t].T @ x[t_chunk, h]; output scaled by 1/sum on
     evacuation (ScalarE/VectorE), DMA out.

bf16 matmul operands / fp32 accumulation end-to-end rel err ~7e-3.
"""

import numpy as np

N_CORES = 8
B_FULL = 64
B_LOC = B_FULL // N_CORES  # 8 batches per core
S = 512
H = 1024
A = 256
P = 128
NT = S // P      # 4 s-tiles per batch
NK = H // P      # 8 h-chunks
NA = A // P      # 2 a-chunks

_CACHE = {}


def _build():
    import concourse.bacc as bacc
    import concourse.mybir as mybir
    import concourse.tile as tile

    F32 = mybir.dt.float32
    BF16 = mybir.dt.bfloat16
    AF = mybir.ActivationFunctionType
    AX = mybir.AxisListType
    ALU = mybir.AluOpType

    nc = bacc.Bacc("TRN2", target_bir_lowering=False, debug=False)

    x_d = nc.dram_tensor("x", [B_LOC, S, H], BF16, kind="ExternalInput").ap()
    xt_d = nc.dram_tensor("xt", [B_LOC, NK, P, S], BF16, kind="ExternalInput").ap()
    wb_d = nc.dram_tensor("wb", [H, A], BF16, kind="ExternalInput").ap()
    u_d = nc.dram_tensor("u4", [P, 2 * NA], BF16, kind="ExternalInput").ap()
    out_d = nc.dram_tensor("out", [B_LOC, H], F32, kind="ExternalOutput").ap()

    with tile.TileContext(nc) as tc:
        with (
            tc.tile_pool(name="const", bufs=1) as cpool,
            tc.tile_pool(name="x", bufs=B_LOC) as xpool,
            tc.tile_pool(name="xt", bufs=5) as xtpool,
            tc.tile_pool(name="ut", bufs=4) as utpool,
            tc.tile_pool(name="sm", bufs=3) as smpool,
            tc.tile_pool(name="o", bufs=3) as opool,
            tc.tile_pool(name="p_u", bufs=4, space="PSUM") as p_u_pool,
            tc.tile_pool(name="p_small", bufs=4, space="PSUM") as p_small_pool,
        ):
            # ---- constants (loaded once) ----
            wb_sb = cpool.tile([P, NK * A], BF16)  # [h_local, (k a)]
            nc.sync.dma_start(
                wb_sb[:].rearrange("p (k a) -> p k a", k=NK),
                wb_d.rearrange("(k p) a -> p k a", p=P),
            )
            u_sb = cpool.tile([P, 2 * NA], BF16)  # [a_local, (a_chunk, zero)]
            nc.sync.dma_start(u_sb[:], u_d[:])
            ones_sb = cpool.tile([P, 1], BF16)
            nc.gpsimd.memset(ones_sb[:], 1.0)

            # PE warm-up overlapping the first xt DMA (HAM un-throttle)
            p_warm = p_u_pool.tile([P, S], F32, tag="p_u")
            for i in range(7):
                nc.tensor.matmul(
                    p_warm[:], wb_sb[:, 0:P], wb_sb[:, 0:S],
                    start=(i == 0), stop=(i == 6),
                )

            def emit_step7(x_sb_, wt_sb_, rs_, b_):
                o_sb = opool.tile([1, H], F32, tag="o_sb")
                for n in range(2):
                    p_o = p_small_pool.tile([1, 512], F32, tag="p_small")
                    for t in range(NT):
                        nc.tensor.matmul(
                            p_o[:],
                            wt_sb_[:, 2 * t : 2 * t + 1],
                            x_sb_[:, t * H + n * 512 : t * H + n * 512 + 512],
                            start=(t == 0),
                            stop=(t == NT - 1),
                        )
                    if n == 0:
                        nc.scalar.activation(
                            o_sb[:, :512], p_o[:], AF.Copy, scale=rs_[:]
                        )
                    else:
                        nc.vector.tensor_scalar_mul(o_sb[:, 512:], p_o[:], rs_[:])
                nc.scalar.dma_start(out_d[b_ : b_ + 1, :], o_sb[:])

            prev = None
            for b in range(B_LOC):
                # ---- 1. load pre-transposed x^T tiles (host layout), then x natural
                # (xt feeds GEMM1 immediately; x natural only needed at step 7,
                #  and rides the scalar HWDGE ring in parallel) ----
                xt_all = xtpool.tile([P, NK * S], BF16, tag="xt")
                if b == 0:
                    for q in range(4):
                        nc.sync.dma_start(
                            xt_all[:, q * 2 * S : (q + 1) * 2 * S].rearrange(
                                "p (k s) -> p k s", k=2
                            ),
                            xt_d[b, 2 * q : 2 * q + 2].rearrange("k p s -> p k s"),
                        )
                elif b == 1:
                    for h2 in range(2):
                        nc.sync.dma_start(
                            xt_all[:, h2 * 4 * S : (h2 + 1) * 4 * S].rearrange(
                                "p (k s) -> p k s", k=4
                            ),
                            xt_d[b, 4 * h2 : 4 * h2 + 4].rearrange("k p s -> p k s"),
                        )
                else:
                    nc.sync.dma_start(
                        xt_all[:].rearrange("p (k s) -> p k s", k=NK),
                        xt_d[b].rearrange("k p s -> p k s"),
                    )
                xt_tiles = [xt_all[:, k * S : (k + 1) * S] for k in range(NK)]
                x_sb = xpool.tile([P, NT * H], BF16, tag="x")
                nc.scalar.dma_start(
                    x_sb[:].rearrange("p (t h) -> p t h", t=NT),
                    x_d[b].rearrange("(t p) h -> p t h", p=P),
                )

                # ---- 3. GEMM1 + tanh -> u_t^T [a_local, s] ----
                ut_tiles = []
                for a in range(NA):
                    p_u = p_u_pool.tile([P, S], F32, tag="p_u")
                    for k in range(NK):
                        nc.tensor.matmul(
                            p_u[:],
                            wb_sb[:, k * A + a * P : k * A + (a + 1) * P],
                            xt_tiles[k],
                            start=(k == 0),
                            stop=(k == NK - 1),
                        )
                    ut_sb = utpool.tile([P, S], BF16, tag="ut")
                    nc.scalar.activation(ut_sb[:], p_u[:], AF.Tanh)
                    ut_tiles.append(ut_sb)

                # ---- 4+5. logit^T [s_local, t] ; wt = exp(logit^T) (bf16) ;
                # softmax sum = ones.T @ wt (even cols) -> 1/sum ----
                p_lt = p_small_pool.tile([P, 2 * NT], F32, tag="p_small")
                for t in range(NT):
                    for a in range(NA):
                        nc.tensor.matmul(
                            p_lt[:, 2 * t : 2 * t + 2],
                            ut_tiles[a][:, t * P : (t + 1) * P],
                            u_sb[:, 2 * a : 2 * a + 2],
                            start=(a == 0),
                            stop=(a == NA - 1),
                        )
                wt_sb = smpool.tile([P, 2 * NT], BF16, tag="wt_sb")
                nc.scalar.activation(wt_sb[:], p_lt[:], AF.Exp)
                p_s = p_small_pool.tile([1, NT], F32, tag="p_small")
                nc.tensor.matmul(
                    p_s[:],
                    ones_sb[:],
                    wt_sb[:].rearrange("p (t two) -> p t two", two=2)[:, :, 0],
                    start=True,
                    stop=True,
                )
                ssum = smpool.tile([1, 1], F32, tag="ssum")
                nc.vector.tensor_reduce(
                    ssum[:], p_s[:], axis=AX.X, op=ALU.add
                )
                rs = smpool.tile([1, 1], F32, tag="rs")
                nc.vector.reciprocal(rs[:], ssum[:])

                # ---- 7. weighted sum (software-pipelined: emitted for the
                # PREVIOUS batch so PE has a full batch of GEMM work between
                # producing wt and consuming it) ----
                pend = (x_sb, wt_sb, rs, b)
                if prev is not None:
                    emit_step7(*prev)
                prev = pend

            emit_step7(*prev)

    nc.compile()
    nc.compile()
    return nc


def get_nc():
    if "nc" not in _CACHE:
        _CACHE["nc"] = _build()
    return _CACHE["nc"]


def make_in_maps(inputs, attention_w, attention_u, attention_b):
    import ml_dtypes

    bf16 = ml_dtypes.bfloat16
    x = np.ascontiguousarray(
        np.asarray(inputs, dtype=np.float32).astype(bf16)
    )
    # pre-transposed layout: xt[b, k, h_local, s] = x[b, s, k*128 + h_local]
    xt = np.ascontiguousarray(
        x.reshape(B_FULL, S, NK, P).transpose(0, 2, 3, 1)
    )
    w = np.asarray(attention_w, dtype=np.float32)
    u = np.asarray(attention_u, dtype=np.float32)
    b = np.asarray(attention_b, dtype=np.float32)
    wb = np.ascontiguousarray(w + b[None, :]).astype(bf16)
    u4 = np.zeros((P, 2 * NA), dtype=np.float32)  # [128, (a_chunk, zero)]
    for a in range(NA):
        u4[:, 2 * a] = u[a * P : (a + 1) * P, 0]
    u4 = u4.astype(bf16)
    in_maps = []
    for c in range(N_CORES):
        in_maps.append(
            {
                "x": x[c * B_LOC : (c + 1) * B_LOC],
                "xt": xt[c * B_LOC : (c + 1) * B_LOC],
                "wb": wb,
                "u4": u4,
            }
        )
    return in_maps


def kernel(inputs, attention_w, attention_u, attention_b):
    from concourse.bass_utils import run_bass_kernel_spmd

    nc = get_nc()
    in_maps = make_in_maps(inputs, attention_w, attention_u, attention_b)
    res = run_bass_kernel_spmd(nc, in_maps, list(range(N_CORES)))
    out = np.concatenate(
        [res.results[c]["out"] for c in range(N_CORES)], axis=0
    ).astype(np.float32)
    return out
